# revision 18
# baseline (speedup 1.0000x reference)
"""LSTM critic kernel for Trainium2 (8 NeuronCores, data-parallel over batch).

Reference computation (per sequence, T=256 steps, hidden H=64):
    gates = [x_t, h] @ W_lstm + b_lstm          # gate order i, j, f, o
    c' = c * sigmoid(f + 1) + sigmoid(i) * tanh(j)
    h' = tanh(c') * sigmoid(o)
    out_t = h' @ W_dec + b_dec

The kernel is latency-bound: the per-step period equals the serial
dependency chain of one chain (h-matmul -> sigmoid -> cell update ->
tanh -> h-mul -> next h-matmul), roughly 2.6us; chains exist only to
keep engines busy inside that latency, and the Activation engine's
per-step busy (one sigmoid + one tanh per chain-step, each with ~185ns
fixed overhead) must stay below the chain latency. NCH=3 balances the
two (ACT ~2.4us busy < ~2.6us chain latency).

Device strategy (per core, batch shard of 512 split into NCH chains):
  - X input (with a constant-1 row carrying the biases) is streamed into
    SBUF in XCHUNK-step windows per chain (one DMA per window, triple
    buffered) — no per-step input DMAs.
  - PSUM tile per chain is PADDED to [128, 1024] fp32 so the two gate
    column groups land in DIFFERENT 2KB psum banks: group1 (o,i) at
    cols [0:CB], group2 (f,j) at cols [512:512+CB]. Accumulation groups
    are per-bank, so BOTH x-part matmuls (K=41, no h dependency) fire
    early, and only the two h-part matmuls (K=64) sit on the critical
    path. One sigmoid ACT op still covers both groups via a 2-free-dim
    access pattern [[512,2],[1,CB]] — same 2*CB element cost.
  - Weights pre-scaled on host so every gate activation is sigmoid(2*x):
      o' = (o + b_o)/2, i' = (i + b_i)/2, f' = (f + b_f + 1)/2, j' = j + b_j
    tanh(j) = 2*sigmoid(2j) - 1 (one cheap tensor_scalar fixup on DVE).
  - One sigmoid ACT op per chain-step over the whole PSUM tile; tanh(c') is
    the only other ACT op (same table set, no reloads). Cell update
    (tj, q, p, c') and the h-mul run on DVE — putting any of them on
    GPSIMD was measured slower end-to-end because Pool ops (~600ns +
    semaphore hops) sit on the c' critical path.
  - h_new is written directly into a rotating [H, OCHUNK*CB] output window
    (double buffered); ONE output DMA per OCHUNK steps per chain on the SP
    HWDGE path. OCHUNK=1 and the [178,167,167] chain split measured best
    in TimelineSim (733,193ns vs 734,370 for OCHUNK=4 / [172,170,170]);
    the HWDGE queue absorbs the extra descriptors off the critical path.
  - (o, f) gates sit at partition base 0 and (i, j) at base 64 because
    walrus requires equal SBUF base partitions for 2-input DVE ops.
  - Decode (hs @ W_dec + b_dec) runs on host over the gathered h output.
"""

import os
import sys

for _p in ("/opt/trn_rl_repo", "/root/.axon_site/_ro/trn_rl_repo"):
    if os.path.isdir(_p) and _p not in sys.path:
        sys.path.insert(0, _p)

import numpy as np

from concourse import bass, mybir, tile
from concourse.bass_utils import run_bass_kernel_spmd

# Problem constants (hardcoded per harness contract).
N, T, OBS, ACT, H = 4096, 256, 32, 8, 64
D = OBS + ACT          # 40
DX = D + 1             # x rows incl. the constant-1 bias row
FORGET_BIAS = 1.0
NCORES = 8
NB = N // NCORES       # 512 sequences per core
SZS = [178, 167, 167]  # chain batch sizes (sum = NB)
NCH = len(SZS)
OFFS = [sum(SZS[:i]) for i in range(NCH + 1)]
XCHUNK = 4             # timesteps per X window DMA
OCHUNK = 1             # timesteps per h output window DMA
PSW = 512              # psum column pitch between the two gate groups

AFT = mybir.ActivationFunctionType
ALU = mybir.AluOpType
BF16 = mybir.dt.bfloat16
F32 = mybir.dt.float32

_BF16_NP = mybir.dt.np(BF16)


def _split_multi_waits(nc, max_waits=1):
    """Workaround for this walrus build's small per-instruction sync-wait
    capacity: hoist excess sem waits onto preceding same-engine NOPs.

    Engines execute in order, so a NOP carrying some of the waits right
    before the real instruction preserves semantics exactly.
    """
    def stale_first(w):
        nm = (w.ant_name or "")
        # DMA / PE / Pool sems are usually stale WAR edges; ACT/DVE sems
        # are usually the live RAW producer edge — keep those on the op.
        return 0 if nm.startswith(("DMA", "PE", "Pool", "SP")) else 1

    for f in nc.m.functions:
        for blk in f.blocks:
            out = []
            changed = False
            for inst in blk.instructions:
                si = inst.sync_info
                if si is not None and si.on_wait and len(si.on_wait) > max_waits:
                    waits = sorted(si.on_wait, key=stale_first)
                    extra, keep = waits[:-max_waits], waits[-max_waits:]
                    for i in range(0, len(extra), max_waits):
                        nop = mybir.InstNoOp(
                            name=f"{inst.name}-wsplit{i}",
                            ins=[],
                            outs=[],
                            engine=inst.engine,
                            sync_info=mybir.SyncInfo(
                                on_wait=extra[i:i + max_waits], on_update=[]
                            ),
                        )
                        out.append(nop)
                    inst.sync_info = mybir.SyncInfo(
                        on_wait=keep, on_update=list(si.on_update)
                    )
                    changed = True
                out.append(inst)
            if changed:
                blk.instructions = out


_ENG_PREFIX = {
    mybir.EngineType.PE: "PE_",
    mybir.EngineType.DVE: "DVE_",
    mybir.EngineType.Activation: "Activation_",
    mybir.EngineType.Pool: "Pool_",
    mybir.EngineType.SP: "SP_",
}


def _drop_same_engine_waits(nc):
    """Remove semaphore waits whose producer ran earlier on the SAME engine.

    Engines execute their instruction stream in order, so a wait on a
    semaphore updated by an earlier instruction of the same engine is
    redundant for sequencing (the data hazard is covered by the engine's
    in-order memory pipeline). Tile-framework sem names are prefixed with
    the producer engine, so the instruction's own engine prefix identifies
    droppable waits. This removes the ~100-200ns sem-propagation +
    wait-split-NOP stall between back-to-back dependent ops on one engine.
    """
    for f in nc.m.functions:
        for blk in f.blocks:
            for inst in blk.instructions:
                si = inst.sync_info
                if si is None or not si.on_wait:
                    continue
                pref = _ENG_PREFIX.get(inst.engine)
                if pref is None:
                    continue
                keep = [
                    w for w in si.on_wait
                    if not (w.ant_name or "").startswith(pref)
                ]
                if len(keep) != len(si.on_wait):
                    inst.sync_info = mybir.SyncInfo(
                        on_wait=keep, on_update=list(si.on_update)
                    )


def _drop_transitive_waits(nc):
    """Drop semaphore waits already implied by an earlier wait on the SAME
    engine.

    Tile sems are monotonic counters and engines execute in order, so once
    an instruction on engine E has waited for sem s >= v, every later
    E-instruction's wait for s >= v' with v' <= v is vacuous. Removing
    them eliminates most wait-split NOPs (each costs ~60-90ns of SEQ time
    right in front of the real instruction).
    """
    for f in nc.m.functions:
        for blk in f.blocks:
            seen = {}
            for inst in blk.instructions:
                si = inst.sync_info
                if si is None or not si.on_wait:
                    continue
                eng = inst.engine
                keep = []
                for w in si.on_wait:
                    if (
                        w.sync_type == "semaphore"
                        and w.wait_mode == "sem-ge-imm"
                        and w.wait_value is not None
                        and w.wait_reg is None
                    ):
                        key = (eng, w.id)
                        if w.wait_value <= seen.get(key, -1):
                            continue
                        seen[key] = w.wait_value
                    keep.append(w)
                if len(keep) != len(si.on_wait):
                    inst.sync_info = mybir.SyncInfo(
                        on_wait=keep, on_update=list(si.on_update)
                    )


def _prep_weights(W_lstm, b_lstm):
    """Split/scale weights into (W1x, W1h, W2x, W2h).

    Gate pre-activations arranged so sigmoid(2*pre) is the right value:
    i, o, f columns halved (f gets +FORGET_BIAS folded), j kept as-is.
    The x-block rows are [W_x | bias]; the bias rides the constant-1 row.
    """
    W = np.asarray(W_lstm, np.float64)
    b = np.asarray(b_lstm, np.float64)
    W_x, W_h = W[:D], W[D:]
    cols = {k: slice(i * H, (i + 1) * H) for i, k in enumerate("ijfo")}

    def blocks(gate, scale, bias_add):
        wx = W_x[:, cols[gate]] * scale
        wh = W_h[:, cols[gate]] * scale
        bb = (b[cols[gate]] + bias_add) * scale
        return np.concatenate([wx, bb[None, :]], axis=0), wh  # [41,64],[64,64]

    xo, ho = blocks("o", 0.5, 0.0)
    xi, hi = blocks("i", 0.5, 0.0)
    xf, hf = blocks("f", 0.5, FORGET_BIAS)
    xj, hj = blocks("j", 1.0, 0.0)
    # Partition-base pairing: (o, f) at psum parts [0:64], (i, j) at [64:128].
    W1x = np.concatenate([xo, xi], axis=1)  # [41, 128]
    W1h = np.concatenate([ho, hi], axis=1)  # [64, 128]
    W2x = np.concatenate([xf, xj], axis=1)
    W2h = np.concatenate([hf, hj], axis=1)
    return W1x, W1h, W2x, W2h


def _build_nc():
    """Build the SPMD bass program (identical on all 8 cores)."""
    nc = bass.Bass()
    X = nc.declare_dram_parameter("x", [T, DX, NB], BF16, isOutput=False)
    W1xd = nc.declare_dram_parameter("w1x", [DX, 2 * H], BF16, isOutput=False)
    W1hd = nc.declare_dram_parameter("w1h", [H, 2 * H], BF16, isOutput=False)
    W2xd = nc.declare_dram_parameter("w2x", [DX, 2 * H], BF16, isOutput=False)
    W2hd = nc.declare_dram_parameter("w2h", [H, 2 * H], BF16, isOutput=False)
    HS = nc.declare_dram_parameter("hs_out", [T, H, NB], BF16, isOutput=True)

    with tile.TileContext(nc) as tc:
        with (
            tc.tile_pool(name="wpool", bufs=1) as wpool,
            tc.tile_pool(name="xw", bufs=3) as xwp,
            tc.tile_pool(name="how", bufs=2) as howp,
            tc.tile_pool(name="ps", bufs=1, space="PSUM") as psp,
            tc.tile_pool(name="sig", bufs=4) as sigp,
            tc.tile_pool(name="small", bufs=6) as smallp,
            tc.tile_pool(name="cst", bufs=4) as cstp,
        ):
            w1x = wpool.tile([DX, 2 * H], BF16, tag="w1x")
            w1h = wpool.tile([H, 2 * H], BF16, tag="w1h")
            w2x = wpool.tile([DX, 2 * H], BF16, tag="w2x")
            w2h = wpool.tile([H, 2 * H], BF16, tag="w2h")
            nc.sync.dma_start(w1x[:], W1xd[:])
            nc.sync.dma_start(w1h[:], W1hd[:])
            nc.sync.dma_start(w2x[:], W2xd[:])
            nc.sync.dma_start(w2h[:], W2hd[:])

            # X windows: per chain, XCHUNK steps per tile, triple-buffered.
            xwin = [{} for _ in range(NCH)]

            def load_xwin(ch, k):
                cb = SZS[ch]
                csl = slice(OFFS[ch], OFFS[ch + 1])
                xt = xwp.tile(
                    [DX, XCHUNK * cb], BF16, tag=f"xw{ch}", name=f"xw{ch}_{k}"
                )
                nc.sync.dma_start(
                    xt[:],
                    X[k * XCHUNK:(k + 1) * XCHUNK, :, csl].rearrange(
                        "t f n -> f t n"
                    ),
                )
                xwin[ch][k] = xt

            for ch in range(NCH):
                load_xwin(ch, 0)
                load_xwin(ch, 1)

            # h output windows: [H, OCHUNK*cb] per chain, double buffered.
            howin = [None] * NCH

            def new_howin(ch, k):
                cb = SZS[ch]
                howin[ch] = howp.tile(
                    [H, OCHUNK * cb], BF16, tag=f"ho{ch}", name=f"ho{ch}_{k}"
                )

            def store_howin(ch, k):
                # DMA window k (steps k*OCHUNK .. k*OCHUNK+OCHUNK-1) out.
                # Rearrange on the DRAM side so the sbuf AP stays [f, t*n]
                # (contiguous cb-sized runs -> wide descriptors).
                csl = slice(OFFS[ch], OFFS[ch + 1])
                nc.sync.dma_start(
                    HS[k * OCHUNK:(k + 1) * OCHUNK, :, csl].rearrange(
                        "t f n -> f t n"
                    ),
                    howin[ch][:],
                )

            h_cur = [None] * NCH
            c_cur = [None] * NCH
            for ch in range(NCH):
                cb = SZS[ch]
                h0 = smallp.tile([H, cb], BF16, tag=f"h0{ch}", bufs=1,
                                 name=f"h{ch}_init")
                nc.vector.memset(h0[:], 0.0)
                c0 = cstp.tile([H, cb], BF16, tag=f"c{ch}", name=f"c{ch}_init")
                nc.vector.memset(c0[:], 0.0)
                h_cur[ch] = h0
                c_cur[ch] = c0
                new_howin(ch, 0)

            def xslice(ch, t):
                cb = SZS[ch]
                tl = t % XCHUNK
                return xwin[ch][t // XCHUNK][:, tl * cb:(tl + 1) * cb]

            def emit_hmul(ch, t, tc_t, s, prio_bump=15):
                # h = tanh(c') * sig(o), written into the output window slice.
                cb = SZS[ch]
                tl = t % OCHUNK
                h_new = howin[ch][:, tl * cb:(tl + 1) * cb]
                bi = nc.vector.tensor_mul(h_new, tc_t[:], s[0:H, 0:cb])
                if prio_bump and getattr(bi.ins, "bass_priority", None) is not None:
                    # Push this op later in the scheduler's priority order so
                    # it does not head-of-line block the lead chain's cell
                    # update on the DVE (it only becomes ready mid-way through
                    # the next step).
                    bi.ins.bass_priority += prio_bump
                h_cur[ch] = h_new

            # The last chain's h-mul is deferred into the next step's emission:
            # in steady state chain NCH-1's tanh lands ~2/3 of a period late,
            # so emitting its h-mul in the current step's DVE stream would
            # head-of-line block the leading chain's next cell update.
            pending_hm = None

            for t in range(T):
                if pending_hm is not None:
                    emit_hmul(*pending_hm, prio_bump=15)
                    pending_hm = None
                if t % XCHUNK == 0:
                    k = t // XCHUNK + 2  # prefetch the window after next
                    if k < T // XCHUNK:
                        for ch in range(NCH):
                            load_xwin(ch, k)
                if t % OCHUNK == 0 and t > 0:
                    for ch in range(NCH):
                        store_howin(ch, t // OCHUNK - 1)
                        new_howin(ch, t // OCHUNK)

                # Phase-grouped emission across chains: engine sequencers
                # stall in-order on semaphore waits, so a waiting op must not
                # have another chain's ready work queued behind it.
                pss, ss, tjs, qs, ps_, cns, tcs = ({} for _ in range(7))
                # Both x-part matmuls fire early: group1 (cols 0:cb) and
                # group2 (cols PSW:PSW+cb) live in different psum banks, so
                # their accumulation groups are independent; each bank's
                # start/stop pair stays consecutive (x then h).
                for ch in range(NCH):
                    cb = SZS[ch]
                    ps = psp.tile(
                        [2 * H, 2 * PSW], F32, tag=f"ps{ch}", name=f"ps{ch}_{t}"
                    )
                    pss[ch] = ps
                    nc.tensor.matmul(
                        ps[:, PSW:PSW + cb], w2x[:], xslice(ch, t),
                        start=True, stop=False,
                    )
                    nc.tensor.matmul(
                        ps[:, 0:cb], w1x[:], xslice(ch, t),
                        start=True, stop=False,
                    )
                # h-part matmuls: the recurrence head, back-to-back per chain.
                for ch in range(NCH):
                    cb = SZS[ch]
                    nc.tensor.matmul(
                        pss[ch][:, PSW:PSW + cb], w2h[:], h_cur[ch][:],
                        start=False, stop=True,
                    )
                    nc.tensor.matmul(
                        pss[ch][:, 0:cb], w1h[:], h_cur[ch][:],
                        start=False, stop=True,
                    )
                for ch in range(NCH):
                    cb = SZS[ch]
                    # S: parts [0:64] = (sig_o | sig_f), [64:128] = (sig_i | sig_2j)
                    # One ACT op over both psum banks via a 2-free-dim AP.
                    s = sigp.tile(
                        [2 * H, 2 * cb], BF16, tag=f"s{ch}", name=f"s{ch}_{t}"
                    )
                    ss[ch] = s
                    pin = pss[ch][:].rearrange("p (g w) -> p g w", g=2)[:, :, 0:cb]
                    sout = s[:].rearrange("p (g w) -> p g w", g=2)
                    nc.scalar.activation(sout, pin, AFT.Sigmoid, scale=2.0)
                for ch in range(NCH):
                    cb = SZS[ch]
                    # Whole cell update per chain back-to-back on DVE (no
                    # cross-waits inside), so each chain's c' lands as early
                    # as possible for its tanh.
                    tj = smallp.tile(
                        [2 * H, cb], BF16, tag=f"tj{ch}", name=f"tj{ch}_{t}"
                    )
                    tjs[ch] = tj
                    nc.vector.tensor_scalar(
                        tj[H:2 * H, :], ss[ch][H:2 * H, cb:2 * cb],
                        2.0, -1.0, ALU.mult, ALU.add,
                    )
                    q = smallp.tile(
                        [H, cb], BF16, tag=f"q{ch}", name=f"q{ch}_{t}"
                    )
                    qs[ch] = q
                    nc.vector.tensor_mul(
                        q[:], c_cur[ch][:], ss[ch][0:H, cb:2 * cb]
                    )
                    p = smallp.tile(
                        [H, cb], BF16, tag=f"p{ch}", name=f"p{ch}_{t}"
                    )
                    ps_[ch] = p
                    nc.vector.tensor_mul(
                        p[:], tjs[ch][H:2 * H, :], ss[ch][H:2 * H, 0:cb]
                    )
                    c_new = cstp.tile(
                        [H, cb], BF16, tag=f"c{ch}", name=f"c{ch}_{t}"
                    )
                    cns[ch] = c_new
                    nc.vector.tensor_add(c_new[:], ps_[ch][:], qs[ch][:])
                    c_cur[ch] = c_new
                    tc_t = smallp.tile(
                        [H, cb], BF16, tag=f"tc{ch}", name=f"tc{ch}_{t}"
                    )
                    tcs[ch] = tc_t
                    nc.scalar.activation(tc_t[:], cns[ch][:], AFT.Tanh)
                    # Interleave h-muls one chain late so each sits in the
                    # DVE stream at its steady-state ready time (chain ch-1's
                    # tanh completes about when chain ch's cell ops issue).
                    if ch >= 1:
                        emit_hmul(ch - 1, t, tcs[ch - 1], ss[ch - 1])
                pending_hm = (NCH - 1, t, tcs[NCH - 1], ss[NCH - 1])

            emit_hmul(*pending_hm)
            # flush the last output window
            for ch in range(NCH):
                store_howin(ch, T // OCHUNK - 1)

    _drop_same_engine_waits(nc)
    _split_multi_waits(nc)
    return nc


_NC_CACHE = None


def _get_nc():
    global _NC_CACHE
    if _NC_CACHE is None:
        _NC_CACHE = _build_nc()
    return _NC_CACHE


def kernel(obss, actions, W_lstm, b_lstm, W_dec, b_dec, _trace=False):
    obss = np.asarray(obss, np.float32)
    actions = np.asarray(actions, np.float32)

    # Host prep: x = [obs | act | 1] in feature-major per-core layout.
    x = np.concatenate(
        [obss, actions, np.ones((N, T, 1), np.float32)], axis=-1
    )  # [N, T, 41]
    W1x, W1h, W2x, W2h = _prep_weights(W_lstm, b_lstm)
    wmaps = {
        "w1x": W1x.astype(_BF16_NP),
        "w1h": W1h.astype(_BF16_NP),
        "w2x": W2x.astype(_BF16_NP),
        "w2h": W2h.astype(_BF16_NP),
    }

    in_maps = []
    for c in range(NCORES):
        xc = np.ascontiguousarray(
            x[c * NB:(c + 1) * NB].transpose(1, 2, 0)
        ).astype(_BF16_NP)  # [T, 41, NB]
        in_maps.append({"x": xc, **wmaps})

    nc = _get_nc()
    res = run_bass_kernel_spmd(nc, in_maps, list(range(NCORES)), trace=_trace)

    # Gather h shards [T, H, NB] -> [T, H, N]; decode on host.
    hs = np.concatenate(
        [res.results[c]["hs_out"].astype(np.float32) for c in range(NCORES)],
        axis=2,
    )
    wd = np.asarray(W_dec, np.float32)[:, 0]
    out = np.einsum("tfn,f->tn", hs, wd) + np.float32(
        np.asarray(b_dec, np.float32)[0]
    )
    out = out[:, :, None].astype(np.float32)  # [T, N, 1]
    if _trace:
        kernel.last_results = res
    return out



# revision 19
# speedup vs baseline: 1.0302x; 1.0302x over previous
"""LSTM critic kernel for Trainium2 (8 NeuronCores, data-parallel over batch).

Reference computation (per sequence, T=256 steps, hidden H=64):
    gates = [x_t, h] @ W_lstm + b_lstm          # gate order i, j, f, o
    c' = c * sigmoid(f + 1) + sigmoid(i) * tanh(j)
    h' = tanh(c') * sigmoid(o)
    out_t = h' @ W_dec + b_dec

The kernel is latency-bound: the per-step period equals the serial
dependency chain of one chain (h-matmul -> sigmoid -> cell update ->
tanh -> h-mul -> next h-matmul), roughly 2.6us; chains exist only to
keep engines busy inside that latency, and the Activation engine's
per-step busy (one sigmoid + one tanh per chain-step, each with ~185ns
fixed overhead) must stay below the chain latency. NCH=3 balances the
two (ACT ~2.4us busy < ~2.6us chain latency).

Device strategy (per core, batch shard of 512 split into NCH chains):
  - X input (with a constant-1 row carrying the biases) is streamed into
    SBUF in XCHUNK-step windows per chain (one DMA per window, triple
    buffered) — no per-step input DMAs.
  - PSUM tile per chain is PADDED to [128, 1024] fp32 so the two gate
    column groups land in DIFFERENT 2KB psum banks: group1 (o,i) at
    cols [0:CB], group2 (f,j) at cols [512:512+CB]. Accumulation groups
    are per-bank, so BOTH x-part matmuls (K=41, no h dependency) fire
    early, and only the two h-part matmuls (K=64) sit on the critical
    path. One sigmoid ACT op still covers both groups via a 2-free-dim
    access pattern [[512,2],[1,CB]] — same 2*CB element cost.
  - Weights pre-scaled on host so every gate activation is sigmoid(2*x):
      o' = (o + b_o)/2, i' = (i + b_i)/2, f' = (f + b_f + 1)/2, j' = j + b_j
    tanh(j) = 2*sigmoid(2j) - 1 (one cheap tensor_scalar fixup on DVE).
  - One sigmoid ACT op per chain-step over the whole PSUM tile; tanh(c') is
    the only other ACT op (same table set, no reloads). Cell update
    (tj, q, p, c') and the h-mul run on DVE — putting any of them on
    GPSIMD was measured slower end-to-end because Pool ops (~600ns +
    semaphore hops) sit on the c' critical path.
  - h_new is written directly into a rotating [H, OCHUNK*CB] output window
    (double buffered); ONE output DMA per OCHUNK steps per chain on the SP
    HWDGE path. OCHUNK=1 and the [178,167,167] chain split measured best
    in TimelineSim (733,193ns vs 734,370 for OCHUNK=4 / [172,170,170]);
    the HWDGE queue absorbs the extra descriptors off the critical path.
  - (o, f) gates sit at partition base 0 and (i, j) at base 64 because
    walrus requires equal SBUF base partitions for 2-input DVE ops.
  - Decode (hs @ W_dec + b_dec) runs on host over the gathered h output.
"""

import os
import sys

for _p in ("/opt/trn_rl_repo", "/root/.axon_site/_ro/trn_rl_repo"):
    if os.path.isdir(_p) and _p not in sys.path:
        sys.path.insert(0, _p)

import numpy as np

from concourse import bass, mybir, tile
from concourse.bass_utils import run_bass_kernel_spmd

# Problem constants (hardcoded per harness contract).
N, T, OBS, ACT, H = 4096, 256, 32, 8, 64
D = OBS + ACT          # 40
DX = D + 1             # x rows incl. the constant-1 bias row
FORGET_BIAS = 1.0
NCORES = 8
NB = N // NCORES       # 512 sequences per core
SZS = [178, 167, 167]  # chain batch sizes (sum = NB)
NCH = len(SZS)
OFFS = [sum(SZS[:i]) for i in range(NCH + 1)]
XCHUNK = 4             # timesteps per X window DMA
OCHUNK = 1             # timesteps per h output window DMA
PSW = 512              # psum column pitch between the two gate groups

AFT = mybir.ActivationFunctionType
ALU = mybir.AluOpType
BF16 = mybir.dt.bfloat16
F32 = mybir.dt.float32

_BF16_NP = mybir.dt.np(BF16)


def _split_multi_waits(nc, max_waits=1):
    """Workaround for this walrus build's small per-instruction sync-wait
    capacity: hoist excess sem waits onto preceding same-engine NOPs.

    Engines execute in order, so a NOP carrying some of the waits right
    before the real instruction preserves semantics exactly.
    """
    def stale_first(w):
        nm = (w.ant_name or "")
        # DMA / PE / Pool sems are usually stale WAR edges; ACT/DVE sems
        # are usually the live RAW producer edge — keep those on the op.
        return 0 if nm.startswith(("DMA", "PE", "Pool", "SP")) else 1

    for f in nc.m.functions:
        for blk in f.blocks:
            out = []
            changed = False
            for inst in blk.instructions:
                si = inst.sync_info
                if si is not None and si.on_wait and len(si.on_wait) > max_waits:
                    waits = sorted(si.on_wait, key=stale_first)
                    extra, keep = waits[:-max_waits], waits[-max_waits:]
                    for i in range(0, len(extra), max_waits):
                        nop = mybir.InstNoOp(
                            name=f"{inst.name}-wsplit{i}",
                            ins=[],
                            outs=[],
                            engine=inst.engine,
                            sync_info=mybir.SyncInfo(
                                on_wait=extra[i:i + max_waits], on_update=[]
                            ),
                        )
                        out.append(nop)
                    inst.sync_info = mybir.SyncInfo(
                        on_wait=keep, on_update=list(si.on_update)
                    )
                    changed = True
                out.append(inst)
            if changed:
                blk.instructions = out


_ENG_PREFIX = {
    mybir.EngineType.PE: "PE_",
    mybir.EngineType.DVE: "DVE_",
    mybir.EngineType.Activation: "Activation_",
    mybir.EngineType.Pool: "Pool_",
    mybir.EngineType.SP: "SP_",
}


def _drop_same_engine_waits(nc):
    """Remove semaphore waits whose producer ran earlier on the SAME engine.

    Engines execute their instruction stream in order, so a wait on a
    semaphore updated by an earlier instruction of the same engine is
    redundant for sequencing (the data hazard is covered by the engine's
    in-order memory pipeline). Tile-framework sem names are prefixed with
    the producer engine, so the instruction's own engine prefix identifies
    droppable waits. This removes the ~100-200ns sem-propagation +
    wait-split-NOP stall between back-to-back dependent ops on one engine.
    """
    for f in nc.m.functions:
        for blk in f.blocks:
            for inst in blk.instructions:
                si = inst.sync_info
                if si is None or not si.on_wait:
                    continue
                pref = _ENG_PREFIX.get(inst.engine)
                if pref is None:
                    continue
                keep = [
                    w for w in si.on_wait
                    if not (w.ant_name or "").startswith(pref)
                ]
                if len(keep) != len(si.on_wait):
                    inst.sync_info = mybir.SyncInfo(
                        on_wait=keep, on_update=list(si.on_update)
                    )


def _drop_transitive_waits(nc):
    """Drop semaphore waits already implied by an earlier wait on the SAME
    engine.

    Tile sems are monotonic counters and engines execute in order, so once
    an instruction on engine E has waited for sem s >= v, every later
    E-instruction's wait for s >= v' with v' <= v is vacuous. Removing
    them eliminates most wait-split NOPs (each costs ~60-90ns of SEQ time
    right in front of the real instruction).
    """
    for f in nc.m.functions:
        for blk in f.blocks:
            seen = {}
            for inst in blk.instructions:
                si = inst.sync_info
                if si is None or not si.on_wait:
                    continue
                eng = inst.engine
                keep = []
                for w in si.on_wait:
                    if (
                        w.sync_type == "semaphore"
                        and w.wait_mode == "sem-ge-imm"
                        and w.wait_value is not None
                        and w.wait_reg is None
                    ):
                        key = (eng, w.id)
                        if w.wait_value <= seen.get(key, -1):
                            continue
                        seen[key] = w.wait_value
                    keep.append(w)
                if len(keep) != len(si.on_wait):
                    inst.sync_info = mybir.SyncInfo(
                        on_wait=keep, on_update=list(si.on_update)
                    )


def _prep_weights(W_lstm, b_lstm):
    """Split/scale weights into (W1x, W1h, W2x, W2h).

    Gate pre-activations arranged so sigmoid(2*pre) is the right value:
    i, o, f columns halved (f gets +FORGET_BIAS folded), j kept as-is.
    The x-block rows are [W_x | bias]; the bias rides the constant-1 row.
    """
    W = np.asarray(W_lstm, np.float64)
    b = np.asarray(b_lstm, np.float64)
    W_x, W_h = W[:D], W[D:]
    cols = {k: slice(i * H, (i + 1) * H) for i, k in enumerate("ijfo")}

    def blocks(gate, scale, bias_add):
        wx = W_x[:, cols[gate]] * scale
        wh = W_h[:, cols[gate]] * scale
        bb = (b[cols[gate]] + bias_add) * scale
        return np.concatenate([wx, bb[None, :]], axis=0), wh  # [41,64],[64,64]

    xo, ho = blocks("o", 0.5, 0.0)
    xi, hi = blocks("i", 0.5, 0.0)
    xf, hf = blocks("f", 0.5, FORGET_BIAS)
    xj, hj = blocks("j", 1.0, 0.0)
    # Partition-base pairing: (o, f) at psum parts [0:64], (i, j) at [64:128].
    W1x = np.concatenate([xo, xi], axis=1)  # [41, 128]
    W1h = np.concatenate([ho, hi], axis=1)  # [64, 128]
    W2x = np.concatenate([xf, xj], axis=1)
    W2h = np.concatenate([hf, hj], axis=1)
    return W1x, W1h, W2x, W2h


def _build_nc():
    """Build the SPMD bass program (identical on all 8 cores)."""
    nc = bass.Bass()
    X = nc.declare_dram_parameter("x", [T, DX, NB], BF16, isOutput=False)
    W1xd = nc.declare_dram_parameter("w1x", [DX, 2 * H], BF16, isOutput=False)
    W1hd = nc.declare_dram_parameter("w1h", [H, 2 * H], BF16, isOutput=False)
    W2xd = nc.declare_dram_parameter("w2x", [DX, 2 * H], BF16, isOutput=False)
    W2hd = nc.declare_dram_parameter("w2h", [H, 2 * H], BF16, isOutput=False)
    HS = nc.declare_dram_parameter("hs_out", [T, H, NB], BF16, isOutput=True)

    with tile.TileContext(nc) as tc:
        with (
            tc.tile_pool(name="wpool", bufs=1) as wpool,
            tc.tile_pool(name="xw", bufs=3) as xwp,
            tc.tile_pool(name="how", bufs=2) as howp,
            tc.tile_pool(name="ps", bufs=1, space="PSUM") as psp,
            tc.tile_pool(name="sig", bufs=4) as sigp,
            tc.tile_pool(name="small", bufs=6) as smallp,
            tc.tile_pool(name="cst", bufs=4) as cstp,
        ):
            w1x = wpool.tile([DX, 2 * H], BF16, tag="w1x")
            w1h = wpool.tile([H, 2 * H], BF16, tag="w1h")
            w2x = wpool.tile([DX, 2 * H], BF16, tag="w2x")
            w2h = wpool.tile([H, 2 * H], BF16, tag="w2h")
            nc.sync.dma_start(w1x[:], W1xd[:])
            nc.sync.dma_start(w1h[:], W1hd[:])
            nc.sync.dma_start(w2x[:], W2xd[:])
            nc.sync.dma_start(w2h[:], W2hd[:])

            # X windows: per chain, XCHUNK steps per tile, triple-buffered.
            xwin = [{} for _ in range(NCH)]

            def load_xwin(ch, k):
                cb = SZS[ch]
                csl = slice(OFFS[ch], OFFS[ch + 1])
                xt = xwp.tile(
                    [DX, XCHUNK * cb], BF16, tag=f"xw{ch}", name=f"xw{ch}_{k}"
                )
                nc.sync.dma_start(
                    xt[:],
                    X[k * XCHUNK:(k + 1) * XCHUNK, :, csl].rearrange(
                        "t f n -> f t n"
                    ),
                )
                xwin[ch][k] = xt

            for ch in range(NCH):
                load_xwin(ch, 0)
                load_xwin(ch, 1)

            # h output windows: [H, OCHUNK*cb] per chain, double buffered.
            howin = [None] * NCH

            def new_howin(ch, k):
                cb = SZS[ch]
                howin[ch] = howp.tile(
                    [H, OCHUNK * cb], BF16, tag=f"ho{ch}", name=f"ho{ch}_{k}"
                )

            def store_howin(ch, k):
                # DMA window k (steps k*OCHUNK .. k*OCHUNK+OCHUNK-1) out.
                # Rearrange on the DRAM side so the sbuf AP stays [f, t*n]
                # (contiguous cb-sized runs -> wide descriptors).
                csl = slice(OFFS[ch], OFFS[ch + 1])
                nc.sync.dma_start(
                    HS[k * OCHUNK:(k + 1) * OCHUNK, :, csl].rearrange(
                        "t f n -> f t n"
                    ),
                    howin[ch][:],
                )

            h_cur = [None] * NCH
            c_cur = [None] * NCH
            for ch in range(NCH):
                cb = SZS[ch]
                h0 = smallp.tile([H, cb], BF16, tag=f"h0{ch}", bufs=1,
                                 name=f"h{ch}_init")
                nc.vector.memset(h0[:], 0.0)
                c0 = cstp.tile([H, cb], BF16, tag=f"c{ch}", name=f"c{ch}_init")
                nc.vector.memset(c0[:], 0.0)
                h_cur[ch] = h0
                c_cur[ch] = c0
                new_howin(ch, 0)

            def xslice(ch, t):
                cb = SZS[ch]
                tl = t % XCHUNK
                return xwin[ch][t // XCHUNK][:, tl * cb:(tl + 1) * cb]

            def emit_hmul(ch, t, tc_t, s, prio_bump=15):
                # h = tanh(c') * sig(o), written into the output window slice.
                cb = SZS[ch]
                tl = t % OCHUNK
                h_new = howin[ch][:, tl * cb:(tl + 1) * cb]
                bi = nc.gpsimd.tensor_mul(h_new, tc_t[:], s[0:H, 0:cb])
                if prio_bump and getattr(bi.ins, "bass_priority", None) is not None:
                    # Push this op later in the scheduler's priority order so
                    # it does not head-of-line block the lead chain's cell
                    # update on the DVE (it only becomes ready mid-way through
                    # the next step).
                    bi.ins.bass_priority += prio_bump
                h_cur[ch] = h_new

            # The last chain's h-mul is deferred into the next step's emission:
            # in steady state chain NCH-1's tanh lands ~2/3 of a period late,
            # so emitting its h-mul in the current step's DVE stream would
            # head-of-line block the leading chain's next cell update.
            pending_hm = None

            for t in range(T):
                if pending_hm is not None:
                    emit_hmul(*pending_hm, prio_bump=15)
                    pending_hm = None
                if t % XCHUNK == 0:
                    k = t // XCHUNK + 2  # prefetch the window after next
                    if k < T // XCHUNK:
                        for ch in range(NCH):
                            load_xwin(ch, k)
                if t % OCHUNK == 0 and t > 0:
                    for ch in range(NCH):
                        store_howin(ch, t // OCHUNK - 1)
                        new_howin(ch, t // OCHUNK)

                # Phase-grouped emission across chains: engine sequencers
                # stall in-order on semaphore waits, so a waiting op must not
                # have another chain's ready work queued behind it.
                pss, ss, tjs, qs, ps_, cns, tcs = ({} for _ in range(7))
                # Both x-part matmuls fire early: group1 (cols 0:cb) and
                # group2 (cols PSW:PSW+cb) live in different psum banks, so
                # their accumulation groups are independent; each bank's
                # start/stop pair stays consecutive (x then h).
                for ch in range(NCH):
                    cb = SZS[ch]
                    ps = psp.tile(
                        [2 * H, 2 * PSW], F32, tag=f"ps{ch}", name=f"ps{ch}_{t}"
                    )
                    pss[ch] = ps
                    nc.tensor.matmul(
                        ps[:, PSW:PSW + cb], w2x[:], xslice(ch, t),
                        start=True, stop=False,
                    )
                    nc.tensor.matmul(
                        ps[:, 0:cb], w1x[:], xslice(ch, t),
                        start=True, stop=False,
                    )
                # h-part matmuls: the recurrence head, back-to-back per chain.
                for ch in range(NCH):
                    cb = SZS[ch]
                    nc.tensor.matmul(
                        pss[ch][:, PSW:PSW + cb], w2h[:], h_cur[ch][:],
                        start=False, stop=True,
                    )
                    nc.tensor.matmul(
                        pss[ch][:, 0:cb], w1h[:], h_cur[ch][:],
                        start=False, stop=True,
                    )
                for ch in range(NCH):
                    cb = SZS[ch]
                    # S: parts [0:64] = (sig_o | sig_f), [64:128] = (sig_i | sig_2j)
                    # One ACT op over both psum banks via a 2-free-dim AP.
                    s = sigp.tile(
                        [2 * H, 2 * cb], BF16, tag=f"s{ch}", name=f"s{ch}_{t}"
                    )
                    ss[ch] = s
                    pin = pss[ch][:].rearrange("p (g w) -> p g w", g=2)[:, :, 0:cb]
                    sout = s[:].rearrange("p (g w) -> p g w", g=2)
                    nc.scalar.activation(sout, pin, AFT.Sigmoid, scale=2.0)
                for ch in range(NCH):
                    cb = SZS[ch]
                    # Whole cell update per chain back-to-back on DVE (no
                    # cross-waits inside), so each chain's c' lands as early
                    # as possible for its tanh.
                    tj = smallp.tile(
                        [2 * H, cb], BF16, tag=f"tj{ch}", name=f"tj{ch}_{t}"
                    )
                    tjs[ch] = tj
                    nc.vector.tensor_scalar(
                        tj[H:2 * H, :], ss[ch][H:2 * H, cb:2 * cb],
                        2.0, -1.0, ALU.mult, ALU.add,
                    )
                    q = smallp.tile(
                        [H, cb], BF16, tag=f"q{ch}", name=f"q{ch}_{t}"
                    )
                    qs[ch] = q
                    nc.vector.tensor_mul(
                        q[:], c_cur[ch][:], ss[ch][0:H, cb:2 * cb]
                    )
                    p = smallp.tile(
                        [H, cb], BF16, tag=f"p{ch}", name=f"p{ch}_{t}"
                    )
                    ps_[ch] = p
                    nc.vector.tensor_mul(
                        p[:], tjs[ch][H:2 * H, :], ss[ch][H:2 * H, 0:cb]
                    )
                    c_new = cstp.tile(
                        [H, cb], BF16, tag=f"c{ch}", name=f"c{ch}_{t}"
                    )
                    cns[ch] = c_new
                    nc.vector.tensor_add(c_new[:], ps_[ch][:], qs[ch][:])
                    c_cur[ch] = c_new
                    tc_t = smallp.tile(
                        [H, cb], BF16, tag=f"tc{ch}", name=f"tc{ch}_{t}"
                    )
                    tcs[ch] = tc_t
                    nc.scalar.activation(tc_t[:], cns[ch][:], AFT.Tanh)
                    # Interleave h-muls one chain late so each sits in the
                    # DVE stream at its steady-state ready time (chain ch-1's
                    # tanh completes about when chain ch's cell ops issue).
                    if ch >= 1:
                        emit_hmul(ch - 1, t, tcs[ch - 1], ss[ch - 1])
                pending_hm = (NCH - 1, t, tcs[NCH - 1], ss[NCH - 1])

            emit_hmul(*pending_hm)
            # flush the last output window
            for ch in range(NCH):
                store_howin(ch, T // OCHUNK - 1)

    _drop_same_engine_waits(nc)
    _split_multi_waits(nc)
    return nc


_NC_CACHE = None


def _get_nc():
    global _NC_CACHE
    if _NC_CACHE is None:
        _NC_CACHE = _build_nc()
    return _NC_CACHE


def kernel(obss, actions, W_lstm, b_lstm, W_dec, b_dec, _trace=False):
    obss = np.asarray(obss, np.float32)
    actions = np.asarray(actions, np.float32)

    # Host prep: x = [obs | act | 1] in feature-major per-core layout.
    x = np.concatenate(
        [obss, actions, np.ones((N, T, 1), np.float32)], axis=-1
    )  # [N, T, 41]
    W1x, W1h, W2x, W2h = _prep_weights(W_lstm, b_lstm)
    wmaps = {
        "w1x": W1x.astype(_BF16_NP),
        "w1h": W1h.astype(_BF16_NP),
        "w2x": W2x.astype(_BF16_NP),
        "w2h": W2h.astype(_BF16_NP),
    }

    in_maps = []
    for c in range(NCORES):
        xc = np.ascontiguousarray(
            x[c * NB:(c + 1) * NB].transpose(1, 2, 0)
        ).astype(_BF16_NP)  # [T, 41, NB]
        in_maps.append({"x": xc, **wmaps})

    nc = _get_nc()
    res = run_bass_kernel_spmd(nc, in_maps, list(range(NCORES)), trace=_trace)

    # Gather h shards [T, H, NB] -> [T, H, N]; decode on host.
    hs = np.concatenate(
        [res.results[c]["hs_out"].astype(np.float32) for c in range(NCORES)],
        axis=2,
    )
    wd = np.asarray(W_dec, np.float32)[:, 0]
    out = np.einsum("tfn,f->tn", hs, wd) + np.float32(
        np.asarray(b_dec, np.float32)[0]
    )
    out = out[:, :, None].astype(np.float32)  # [T, N, 1]
    if _trace:
        kernel.last_results = res
    return out



# revision 20
# speedup vs baseline: 1.0332x; 1.0030x over previous
"""LSTM critic kernel for Trainium2 (8 NeuronCores, data-parallel over batch).

Reference computation (per sequence, T=256 steps, hidden H=64):
    gates = [x_t, h] @ W_lstm + b_lstm          # gate order i, j, f, o
    c' = c * sigmoid(f + 1) + sigmoid(i) * tanh(j)
    h' = tanh(c') * sigmoid(o)
    out_t = h' @ W_dec + b_dec

The kernel is latency-bound: the per-step period equals the serial
dependency chain of one chain (h-matmul -> sigmoid -> cell update ->
tanh -> h-mul -> next h-matmul), roughly 2.6us; chains exist only to
keep engines busy inside that latency, and the Activation engine's
per-step busy (one sigmoid + one tanh per chain-step, each with ~185ns
fixed overhead) must stay below the chain latency. NCH=3 balances the
two (ACT ~2.4us busy < ~2.6us chain latency).

Device strategy (per core, batch shard of 512 split into NCH chains):
  - X input (with a constant-1 row carrying the biases) is streamed into
    SBUF in XCHUNK-step windows per chain (one DMA per window, triple
    buffered) — no per-step input DMAs.
  - PSUM tile per chain is PADDED to [128, 1024] fp32 so the two gate
    column groups land in DIFFERENT 2KB psum banks: group1 (o,i) at
    cols [0:CB], group2 (f,j) at cols [512:512+CB]. Accumulation groups
    are per-bank, so BOTH x-part matmuls (K=41, no h dependency) fire
    early, and only the two h-part matmuls (K=64) sit on the critical
    path. One sigmoid ACT op still covers both groups via a 2-free-dim
    access pattern [[512,2],[1,CB]] — same 2*CB element cost.
  - Weights pre-scaled on host so every gate activation is sigmoid(2*x):
      o' = (o + b_o)/2, i' = (i + b_i)/2, f' = (f + b_f + 1)/2, j' = j + b_j
    tanh(j) = 2*sigmoid(2j) - 1 (one cheap tensor_scalar fixup on DVE).
  - One sigmoid ACT op per chain-step over the whole PSUM tile; tanh(c') is
    the only other ACT op (same table set, no reloads). Cell update
    (tj, q, p, c') runs on DVE; the h-mul runs on the Pool/GPSIMD
    engine. At the moment each chain's c' and another chain's h-mul
    become ready, they used to serialize on DVE (~150ns added to the
    tanh data-path every step); Pool is slower per op (~430ns vs 150)
    but each chain's next sigmoid has engine-bound slack that absorbs
    it, and freeing DVE lets c' land before ACT goes idle (-21.5us).
    q/p/tj must stay on DVE (Pool there costs 50-170us).
  - h_new is written directly into a rotating [H, OCHUNK*CB] output window
    (double buffered); ONE output DMA per OCHUNK steps per chain on the SP
    HWDGE path. OCHUNK=1 and the [178,167,167] chain split measured best
    in TimelineSim (733,193ns vs 734,370 for OCHUNK=4 / [172,170,170]);
    the HWDGE queue absorbs the extra descriptors off the critical path.
  - (o, f) gates sit at partition base 0 and (i, j) at base 64 because
    walrus requires equal SBUF base partitions for 2-input DVE ops.
  - Decode (hs @ W_dec + b_dec) runs on host over the gathered h output.
"""

import os
import sys

for _p in ("/opt/trn_rl_repo", "/root/.axon_site/_ro/trn_rl_repo"):
    if os.path.isdir(_p) and _p not in sys.path:
        sys.path.insert(0, _p)

import numpy as np

from concourse import bass, mybir, tile
from concourse.bass_utils import run_bass_kernel_spmd

# Problem constants (hardcoded per harness contract).
N, T, OBS, ACT, H = 4096, 256, 32, 8, 64
D = OBS + ACT          # 40
DX = D + 1             # x rows incl. the constant-1 bias row
FORGET_BIAS = 1.0
NCORES = 8
NB = N // NCORES       # 512 sequences per core
SZS = [176, 168, 168]  # chain batch sizes (sum = NB)
NCH = len(SZS)
OFFS = [sum(SZS[:i]) for i in range(NCH + 1)]
XCHUNK = 4             # timesteps per X window DMA
OCHUNK = 1             # timesteps per h output window DMA
PSW = 512              # psum column pitch between the two gate groups

AFT = mybir.ActivationFunctionType
ALU = mybir.AluOpType
BF16 = mybir.dt.bfloat16
F32 = mybir.dt.float32

_BF16_NP = mybir.dt.np(BF16)


def _split_multi_waits(nc, max_waits=1):
    """Workaround for this walrus build's small per-instruction sync-wait
    capacity: hoist excess sem waits onto preceding same-engine NOPs.

    Engines execute in order, so a NOP carrying some of the waits right
    before the real instruction preserves semantics exactly.
    """
    def stale_first(w):
        nm = (w.ant_name or "")
        # DMA / PE / Pool sems are usually stale WAR edges; ACT/DVE sems
        # are usually the live RAW producer edge — keep those on the op.
        return 0 if nm.startswith(("DMA", "PE", "Pool", "SP")) else 1

    for f in nc.m.functions:
        for blk in f.blocks:
            out = []
            changed = False
            for inst in blk.instructions:
                si = inst.sync_info
                if si is not None and si.on_wait and len(si.on_wait) > max_waits:
                    waits = sorted(si.on_wait, key=stale_first)
                    extra, keep = waits[:-max_waits], waits[-max_waits:]
                    for i in range(0, len(extra), max_waits):
                        nop = mybir.InstNoOp(
                            name=f"{inst.name}-wsplit{i}",
                            ins=[],
                            outs=[],
                            engine=inst.engine,
                            sync_info=mybir.SyncInfo(
                                on_wait=extra[i:i + max_waits], on_update=[]
                            ),
                        )
                        out.append(nop)
                    inst.sync_info = mybir.SyncInfo(
                        on_wait=keep, on_update=list(si.on_update)
                    )
                    changed = True
                out.append(inst)
            if changed:
                blk.instructions = out


_ENG_PREFIX = {
    mybir.EngineType.PE: "PE_",
    mybir.EngineType.DVE: "DVE_",
    mybir.EngineType.Activation: "Activation_",
    mybir.EngineType.Pool: "Pool_",
    mybir.EngineType.SP: "SP_",
}


def _drop_same_engine_waits(nc):
    """Remove semaphore waits whose producer ran earlier on the SAME engine.

    Engines execute their instruction stream in order, so a wait on a
    semaphore updated by an earlier instruction of the same engine is
    redundant for sequencing (the data hazard is covered by the engine's
    in-order memory pipeline). Tile-framework sem names are prefixed with
    the producer engine, so the instruction's own engine prefix identifies
    droppable waits. This removes the ~100-200ns sem-propagation +
    wait-split-NOP stall between back-to-back dependent ops on one engine.
    """
    for f in nc.m.functions:
        for blk in f.blocks:
            for inst in blk.instructions:
                si = inst.sync_info
                if si is None or not si.on_wait:
                    continue
                pref = _ENG_PREFIX.get(inst.engine)
                if pref is None:
                    continue
                keep = [
                    w for w in si.on_wait
                    if not (w.ant_name or "").startswith(pref)
                ]
                if len(keep) != len(si.on_wait):
                    inst.sync_info = mybir.SyncInfo(
                        on_wait=keep, on_update=list(si.on_update)
                    )


def _drop_transitive_waits(nc):
    """Drop semaphore waits already implied by an earlier wait on the SAME
    engine.

    Tile sems are monotonic counters and engines execute in order, so once
    an instruction on engine E has waited for sem s >= v, every later
    E-instruction's wait for s >= v' with v' <= v is vacuous. Removing
    them eliminates most wait-split NOPs (each costs ~60-90ns of SEQ time
    right in front of the real instruction).
    """
    for f in nc.m.functions:
        for blk in f.blocks:
            seen = {}
            for inst in blk.instructions:
                si = inst.sync_info
                if si is None or not si.on_wait:
                    continue
                eng = inst.engine
                keep = []
                for w in si.on_wait:
                    if (
                        w.sync_type == "semaphore"
                        and w.wait_mode == "sem-ge-imm"
                        and w.wait_value is not None
                        and w.wait_reg is None
                    ):
                        key = (eng, w.id)
                        if w.wait_value <= seen.get(key, -1):
                            continue
                        seen[key] = w.wait_value
                    keep.append(w)
                if len(keep) != len(si.on_wait):
                    inst.sync_info = mybir.SyncInfo(
                        on_wait=keep, on_update=list(si.on_update)
                    )


def _prep_weights(W_lstm, b_lstm):
    """Split/scale weights into (W1x, W1h, W2x, W2h).

    Gate pre-activations arranged so sigmoid(2*pre) is the right value:
    i, o, f columns halved (f gets +FORGET_BIAS folded), j kept as-is.
    The x-block rows are [W_x | bias]; the bias rides the constant-1 row.
    """
    W = np.asarray(W_lstm, np.float64)
    b = np.asarray(b_lstm, np.float64)
    W_x, W_h = W[:D], W[D:]
    cols = {k: slice(i * H, (i + 1) * H) for i, k in enumerate("ijfo")}

    def blocks(gate, scale, bias_add):
        wx = W_x[:, cols[gate]] * scale
        wh = W_h[:, cols[gate]] * scale
        bb = (b[cols[gate]] + bias_add) * scale
        return np.concatenate([wx, bb[None, :]], axis=0), wh  # [41,64],[64,64]

    xo, ho = blocks("o", 0.5, 0.0)
    xi, hi = blocks("i", 0.5, 0.0)
    xf, hf = blocks("f", 0.5, FORGET_BIAS)
    xj, hj = blocks("j", 1.0, 0.0)
    # Partition-base pairing: (o, f) at psum parts [0:64], (i, j) at [64:128].
    W1x = np.concatenate([xo, xi], axis=1)  # [41, 128]
    W1h = np.concatenate([ho, hi], axis=1)  # [64, 128]
    W2x = np.concatenate([xf, xj], axis=1)
    W2h = np.concatenate([hf, hj], axis=1)
    return W1x, W1h, W2x, W2h


def _build_nc():
    """Build the SPMD bass program (identical on all 8 cores)."""
    nc = bass.Bass()
    X = nc.declare_dram_parameter("x", [T, DX, NB], BF16, isOutput=False)
    W1xd = nc.declare_dram_parameter("w1x", [DX, 2 * H], BF16, isOutput=False)
    W1hd = nc.declare_dram_parameter("w1h", [H, 2 * H], BF16, isOutput=False)
    W2xd = nc.declare_dram_parameter("w2x", [DX, 2 * H], BF16, isOutput=False)
    W2hd = nc.declare_dram_parameter("w2h", [H, 2 * H], BF16, isOutput=False)
    HS = nc.declare_dram_parameter("hs_out", [T, H, NB], BF16, isOutput=True)

    with tile.TileContext(nc) as tc:
        with (
            tc.tile_pool(name="wpool", bufs=1) as wpool,
            tc.tile_pool(name="xw", bufs=3) as xwp,
            tc.tile_pool(name="how", bufs=2) as howp,
            tc.tile_pool(name="ps", bufs=1, space="PSUM") as psp,
            tc.tile_pool(name="sig", bufs=4) as sigp,
            tc.tile_pool(name="small", bufs=6) as smallp,
            tc.tile_pool(name="cst", bufs=4) as cstp,
        ):
            w1x = wpool.tile([DX, 2 * H], BF16, tag="w1x")
            w1h = wpool.tile([H, 2 * H], BF16, tag="w1h")
            w2x = wpool.tile([DX, 2 * H], BF16, tag="w2x")
            w2h = wpool.tile([H, 2 * H], BF16, tag="w2h")
            nc.sync.dma_start(w1x[:], W1xd[:])
            nc.sync.dma_start(w1h[:], W1hd[:])
            nc.sync.dma_start(w2x[:], W2xd[:])
            nc.sync.dma_start(w2h[:], W2hd[:])

            # X windows: per chain, XCHUNK steps per tile, triple-buffered.
            xwin = [{} for _ in range(NCH)]

            def load_xwin(ch, k):
                cb = SZS[ch]
                csl = slice(OFFS[ch], OFFS[ch + 1])
                xt = xwp.tile(
                    [DX, XCHUNK * cb], BF16, tag=f"xw{ch}", name=f"xw{ch}_{k}"
                )
                nc.sync.dma_start(
                    xt[:],
                    X[k * XCHUNK:(k + 1) * XCHUNK, :, csl].rearrange(
                        "t f n -> f t n"
                    ),
                )
                xwin[ch][k] = xt

            for ch in range(NCH):
                load_xwin(ch, 0)
                load_xwin(ch, 1)

            # h output windows: [H, OCHUNK*cb] per chain, double buffered.
            howin = [None] * NCH

            def new_howin(ch, k):
                cb = SZS[ch]
                howin[ch] = howp.tile(
                    [H, OCHUNK * cb], BF16, tag=f"ho{ch}", name=f"ho{ch}_{k}"
                )

            def store_howin(ch, k):
                # DMA window k (steps k*OCHUNK .. k*OCHUNK+OCHUNK-1) out.
                # Rearrange on the DRAM side so the sbuf AP stays [f, t*n]
                # (contiguous cb-sized runs -> wide descriptors).
                csl = slice(OFFS[ch], OFFS[ch + 1])
                nc.sync.dma_start(
                    HS[k * OCHUNK:(k + 1) * OCHUNK, :, csl].rearrange(
                        "t f n -> f t n"
                    ),
                    howin[ch][:],
                )

            h_cur = [None] * NCH
            c_cur = [None] * NCH
            for ch in range(NCH):
                cb = SZS[ch]
                h0 = smallp.tile([H, cb], BF16, tag=f"h0{ch}", bufs=1,
                                 name=f"h{ch}_init")
                nc.vector.memset(h0[:], 0.0)
                c0 = cstp.tile([H, cb], BF16, tag=f"c{ch}", name=f"c{ch}_init")
                nc.vector.memset(c0[:], 0.0)
                h_cur[ch] = h0
                c_cur[ch] = c0
                new_howin(ch, 0)

            def xslice(ch, t):
                cb = SZS[ch]
                tl = t % XCHUNK
                return xwin[ch][t // XCHUNK][:, tl * cb:(tl + 1) * cb]

            def emit_hmul(ch, t, tc_t, s, prio_bump=15):
                # h = tanh(c') * sig(o), written into the output window slice.
                cb = SZS[ch]
                tl = t % OCHUNK
                h_new = howin[ch][:, tl * cb:(tl + 1) * cb]
                bi = nc.gpsimd.tensor_mul(h_new, tc_t[:], s[0:H, 0:cb])
                if prio_bump and getattr(bi.ins, "bass_priority", None) is not None:
                    # Push this op later in the scheduler's priority order so
                    # it does not head-of-line block the lead chain's cell
                    # update on the DVE (it only becomes ready mid-way through
                    # the next step).
                    bi.ins.bass_priority += prio_bump
                h_cur[ch] = h_new

            # The last chain's h-mul is deferred into the next step's emission:
            # in steady state chain NCH-1's tanh lands ~2/3 of a period late,
            # so emitting its h-mul in the current step's DVE stream would
            # head-of-line block the leading chain's next cell update.
            pending_hm = None

            for t in range(T):
                if pending_hm is not None:
                    emit_hmul(*pending_hm, prio_bump=15)
                    pending_hm = None
                if t % XCHUNK == 0:
                    k = t // XCHUNK + 2  # prefetch the window after next
                    if k < T // XCHUNK:
                        for ch in range(NCH):
                            load_xwin(ch, k)
                if t % OCHUNK == 0 and t > 0:
                    for ch in range(NCH):
                        store_howin(ch, t // OCHUNK - 1)
                        new_howin(ch, t // OCHUNK)

                # Phase-grouped emission across chains: engine sequencers
                # stall in-order on semaphore waits, so a waiting op must not
                # have another chain's ready work queued behind it.
                pss, ss, tjs, qs, ps_, cns, tcs = ({} for _ in range(7))
                # Both x-part matmuls fire early: group1 (cols 0:cb) and
                # group2 (cols PSW:PSW+cb) live in different psum banks, so
                # their accumulation groups are independent; each bank's
                # start/stop pair stays consecutive (x then h).
                for ch in range(NCH):
                    cb = SZS[ch]
                    ps = psp.tile(
                        [2 * H, 2 * PSW], F32, tag=f"ps{ch}", name=f"ps{ch}_{t}"
                    )
                    pss[ch] = ps
                    nc.tensor.matmul(
                        ps[:, PSW:PSW + cb], w2x[:], xslice(ch, t),
                        start=True, stop=False,
                    )
                    nc.tensor.matmul(
                        ps[:, 0:cb], w1x[:], xslice(ch, t),
                        start=True, stop=False,
                    )
                # h-part matmuls: the recurrence head, back-to-back per chain.
                for ch in range(NCH):
                    cb = SZS[ch]
                    nc.tensor.matmul(
                        pss[ch][:, PSW:PSW + cb], w2h[:], h_cur[ch][:],
                        start=False, stop=True,
                    )
                    nc.tensor.matmul(
                        pss[ch][:, 0:cb], w1h[:], h_cur[ch][:],
                        start=False, stop=True,
                    )
                for ch in range(NCH):
                    cb = SZS[ch]
                    # S: parts [0:64] = (sig_o | sig_f), [64:128] = (sig_i | sig_2j)
                    # One ACT op over both psum banks via a 2-free-dim AP.
                    s = sigp.tile(
                        [2 * H, 2 * cb], BF16, tag=f"s{ch}", name=f"s{ch}_{t}"
                    )
                    ss[ch] = s
                    pin = pss[ch][:].rearrange("p (g w) -> p g w", g=2)[:, :, 0:cb]
                    sout = s[:].rearrange("p (g w) -> p g w", g=2)
                    nc.scalar.activation(sout, pin, AFT.Sigmoid, scale=2.0)
                for ch in range(NCH):
                    cb = SZS[ch]
                    # Whole cell update per chain back-to-back on DVE (no
                    # cross-waits inside), so each chain's c' lands as early
                    # as possible for its tanh.
                    tj = smallp.tile(
                        [2 * H, cb], BF16, tag=f"tj{ch}", name=f"tj{ch}_{t}"
                    )
                    tjs[ch] = tj
                    nc.vector.tensor_scalar(
                        tj[H:2 * H, :], ss[ch][H:2 * H, cb:2 * cb],
                        2.0, -1.0, ALU.mult, ALU.add,
                    )
                    q = smallp.tile(
                        [H, cb], BF16, tag=f"q{ch}", name=f"q{ch}_{t}"
                    )
                    qs[ch] = q
                    nc.vector.tensor_mul(
                        q[:], c_cur[ch][:], ss[ch][0:H, cb:2 * cb]
                    )
                    p = smallp.tile(
                        [H, cb], BF16, tag=f"p{ch}", name=f"p{ch}_{t}"
                    )
                    ps_[ch] = p
                    nc.vector.tensor_mul(
                        p[:], tjs[ch][H:2 * H, :], ss[ch][H:2 * H, 0:cb]
                    )
                    c_new = cstp.tile(
                        [H, cb], BF16, tag=f"c{ch}", name=f"c{ch}_{t}"
                    )
                    cns[ch] = c_new
                    nc.vector.tensor_add(c_new[:], ps_[ch][:], qs[ch][:])
                    c_cur[ch] = c_new
                    tc_t = smallp.tile(
                        [H, cb], BF16, tag=f"tc{ch}", name=f"tc{ch}_{t}"
                    )
                    tcs[ch] = tc_t
                    nc.scalar.activation(tc_t[:], cns[ch][:], AFT.Tanh)
                    # Interleave h-muls one chain late so each sits in the
                    # DVE stream at its steady-state ready time (chain ch-1's
                    # tanh completes about when chain ch's cell ops issue).
                    if ch >= 1:
                        emit_hmul(ch - 1, t, tcs[ch - 1], ss[ch - 1])
                pending_hm = (NCH - 1, t, tcs[NCH - 1], ss[NCH - 1])

            emit_hmul(*pending_hm)
            # flush the last output window
            for ch in range(NCH):
                store_howin(ch, T // OCHUNK - 1)

    _drop_same_engine_waits(nc)
    _split_multi_waits(nc)
    return nc


_NC_CACHE = None


def _get_nc():
    global _NC_CACHE
    if _NC_CACHE is None:
        _NC_CACHE = _build_nc()
    return _NC_CACHE


def kernel(obss, actions, W_lstm, b_lstm, W_dec, b_dec, _trace=False):
    obss = np.asarray(obss, np.float32)
    actions = np.asarray(actions, np.float32)

    # Host prep: x = [obs | act | 1] in feature-major per-core layout.
    x = np.concatenate(
        [obss, actions, np.ones((N, T, 1), np.float32)], axis=-1
    )  # [N, T, 41]
    W1x, W1h, W2x, W2h = _prep_weights(W_lstm, b_lstm)
    wmaps = {
        "w1x": W1x.astype(_BF16_NP),
        "w1h": W1h.astype(_BF16_NP),
        "w2x": W2x.astype(_BF16_NP),
        "w2h": W2h.astype(_BF16_NP),
    }

    in_maps = []
    for c in range(NCORES):
        xc = np.ascontiguousarray(
            x[c * NB:(c + 1) * NB].transpose(1, 2, 0)
        ).astype(_BF16_NP)  # [T, 41, NB]
        in_maps.append({"x": xc, **wmaps})

    nc = _get_nc()
    res = run_bass_kernel_spmd(nc, in_maps, list(range(NCORES)), trace=_trace)

    # Gather h shards [T, H, NB] -> [T, H, N]; decode on host.
    hs = np.concatenate(
        [res.results[c]["hs_out"].astype(np.float32) for c in range(NCORES)],
        axis=2,
    )
    wd = np.asarray(W_dec, np.float32)[:, 0]
    out = np.einsum("tfn,f->tn", hs, wd) + np.float32(
        np.asarray(b_dec, np.float32)[0]
    )
    out = out[:, :, None].astype(np.float32)  # [T, N, 1]
    if _trace:
        kernel.last_results = res
    return out

